# revision 2
# baseline (speedup 1.0000x reference)
"""Trainium2 Bass kernel for MoEAdaptorLayer (moe_routing).

Reference computation (B=512, L=50, D=768, O=300, E=8):
    gates = softmax(x @ w_gate)                          # [B,L,E]
    xw    = einsum('bli,eoi->bleo', x, expert_w)         # [B,L,E,O]
    bw    = einsum('eli,eoi->leo', expert_bias, expert_w)
    out   = einsum('ble,bleo->blo', gates, xw - bw[None])

Strategy: data-parallel over B across 8 cores (64 batches/core); no
collectives. Tokens are laid out l-major per core (token = l*64 + b), so each
128-token tile covers exactly two l values. Matmul operands are rounded to
fp16 on device (fp32 PSUM accumulation; ~3e-4 relative error) — fp16 streams
at full PE rate where fp32 runs at 1/4. Per 128-token tile:
  - one contiguous 393KB DMA of host-pre-transposed x, rounded to fp16 on ACT;
  - 6 K-chunk matmuls per expert, chunk-major so the stationary operand is
    shared; gate-logit columns are packed ahead of expert 0's weights so one
    matmul stream computes [gates | expert0] into one PSUM bank;
  - softmax on ACT/DVE; normalized gates are transposed via the PE so the
    gate-weighted bias correction  -sum_e g_e bw[l,e,:]  is two tiny K=8
    matmuls into PSUM (one per l-half);
  - the 8 expert outputs are folded as acc = sum_e g_e * P_e + corr by a
    vector-engine scalar_tensor_tensor chain whose first op reads the corr
    PSUM bank and whose last op writes the DMA-out tile directly.
"""

import sys

sys.path.insert(0, "/opt/trn_rl_repo")

from contextlib import ExitStack

import numpy as np

import concourse.bass as bass  # noqa: F401
import concourse.tile as tile
from concourse import bacc, mybir
from concourse import bass_utils
from concourse.masks import make_identity

B, L, D, O, E = 512, 50, 768, 300, 8
NCORES = 8
BC = B // NCORES          # 64 batches per core
TOK = BC * L              # 3200 tokens per core
P = 128                   # tokens per tile
NT = TOK // P             # 25 tiles per core
KC = D // 128             # 6 contraction chunks
WCOL = E + E * O          # packed w row: [gate(8) | e0(300) | ... | e7(300)]

F32 = mybir.dt.float32
FP16 = mybir.dt.float16

_CACHE: dict = {}


def _build_nc(reps: int = 1, mmdt=FP16):
    nc = bacc.Bacc("TRN2", target_bir_lowering=False, debug=False,
                   num_devices=NCORES)

    xt_d = nc.dram_tensor("xt", [NT, P, KC, 128], FP16, kind="ExternalInput").ap()
    w_d = nc.dram_tensor("w", [128, KC, WCOL], FP16, kind="ExternalInput").ap()
    bias_d = nc.dram_tensor("bias", [128, KC, E, L], FP16, kind="ExternalInput").ap()
    out_d = nc.dram_tensor("out", [NT, P, O], F32, kind="ExternalOutput").ap()

    with tile.TileContext(nc) as tc, ExitStack() as ctx:
        const = ctx.enter_context(tc.tile_pool(name="const", bufs=1))
        stage = ctx.enter_context(tc.tile_pool(name="stage", bufs=2))
        xpool = ctx.enter_context(tc.tile_pool(name="xpool", bufs=3))
        spool = ctx.enter_context(tc.tile_pool(name="spool", bufs=3))
        opool = ctx.enter_context(tc.tile_pool(name="opool", bufs=3))
        dpool = ctx.enter_context(tc.tile_pool(name="dram", bufs=1, space="DRAM"))
        pexp = ctx.enter_context(tc.tile_pool(name="pexp", bufs=7, space="PSUM"))
        pcor = ctx.enter_context(tc.tile_pool(name="pcor", bufs=1, space="PSUM"))

        # --- Phase 0: params (host-pre-rounded fp16, no staging copies) ---
        # pre-issue tile-0 x DMA so it's resident when the PE ramps up
        xr0 = xpool.tile([P, KC, 128], FP16, tag="xr", name="xr_pre0")
        nc.sync.dma_start(xr0[:], xt_d[0])

        w_sb, bias_sb = [], []
        for c in range(KC):
            wc = const.tile([128, WCOL], FP16, tag=f"w{c}", name=f"w_sb{c}")
            nc.sync.dma_start(wc[:], w_d[:, c])
            w_sb.append(wc)
            bc = const.tile([128, E, L], FP16, tag=f"b{c}", name=f"bias_sb{c}")
            nc.sync.dma_start(bc[:], bias_d[:, c])
            bias_sb.append(bc)

        ident = const.tile([128, 128], F32, tag="ident")
        make_identity(nc, ident[:])

        # negbw[e, l*O+o] = -sum_i expert_bias[e,l,i] * expert_w[e,o,i],
        # e on partitions (rhs layout for the tiny corr matmuls).
        # Computed [l, o] per expert in PSUM, negated+rounded to fp16,
        # relaid out through a DRAM bounce.
        negbwT = const.tile([E, L * O], FP16, tag="negbwT")
        scratch = dpool.tile([E, L, O], FP16, tag="nbscratch")

        def emit_negbw():
            for e in range(E):
                pbw = pexp.tile([L, O], F32, tag="pexp", name=f"pbw{e}")
                for c in range(KC):
                    nc.tensor.matmul(pbw[:], bias_sb[c][:, e, :],
                                     w_sb[c][:, E + e * O:E + (e + 1) * O],
                                     start=(c == 0), stop=(c == KC - 1))
                nbst = stage.tile([L, O], FP16, tag="nbst", name=f"nbst{e}")
                nc.scalar.mul(nbst[:], pbw[:], -1.0)
                nc.sync.dma_start(scratch[e], nbst[:])
            nc.sync.dma_start(negbwT[:],
                              scratch[:].rearrange("e l o -> e (l o)"))

        # --- Phase 1: token tiles ----------------------------------------
        def tile_state(rep, t, xr=None):
            if xr is None:
                xr = xpool.tile([P, KC, 128], FP16, tag="xr",
                                name=f"xr{rep}_{t}")
                nc.sync.dma_start(xr[:], xt_d[t])
            return {
                "xr": xr,
                "gexp": spool.tile([P, E], F32, tag="gexp", name=f"gexp{rep}_{t}"),
                "gsum": spool.tile([P, 1], F32, tag="gsum", name=f"gsum{rep}_{t}"),
                "rs": spool.tile([P, 1], F32, tag="rs", name=f"rs{rep}_{t}"),
                "gn": spool.tile([P, E], F32, tag="gn", name=f"gn{rep}_{t}"),
                "gts": spool.tile([E, P], FP16, tag="gts", name=f"gts{rep}_{t}"),
                "acc": spool.tile([P, O], F32, tag="acc", name=f"acc{rep}_{t}"),
                "osb": opool.tile([P, O], F32, tag="osb", name=f"osb{rep}_{t}"),
            }

        def emit_group_mm(rep, t, st, g):
            xr = st["xr"]
            pes = []
            for j in range(4):
                e = 4 * g + j
                wid = O + E if e == 0 else O
                pes.append(pexp.tile([P, wid], F32, tag="pexp",
                                     name=f"pe{rep}_{t}_{g}_{j}"))
            for c in range(KC):
                for j in range(4):
                    e = 4 * g + j
                    lo = 0 if e == 0 else E + e * O
                    nc.tensor.matmul(pes[j][:], xr[:, c, :],
                                     w_sb[c][:, lo:lo + pes[j].shape[-1]],
                                     start=(c == 0), stop=(c == KC - 1))
            return pes

        def emit_group_tail(rep, t, st, g, pes, ptr):
            gexp, gsum, rs, gn = st["gexp"], st["gsum"], st["rs"], st["gn"]
            acc, osb = st["acc"], st["osb"]
            if g == 0:
                # softmax without max-subtraction (|logits| <~ 3 here);
                # gate logits live in cols 0:8 of expert-0's PSUM bank
                nc.scalar.activation(gexp[:], pes[0][:, 0:E],
                                     mybir.ActivationFunctionType.Exp,
                                     accum_out=gsum[:])
                nc.vector.reciprocal(rs[:], gsum[:])
                nc.vector.tensor_scalar_mul(gn[:], gexp[:], rs[:])
                # gate-weighted bias correction:
                #   corr[m, :] = -sum_e gn[m,e] * bw[l(m), e, :]
                # via gn^T (PE transpose) and two K=8 matmuls, one per l-half
                nc.tensor.transpose(ptr[0:E, 0:P], gn[:], ident[:])
                nc.vector.tensor_copy(st["gts"][:], ptr[0:E, 0:P])
                for h in range(2):
                    lt = 2 * t + h
                    nc.tensor.matmul(ptr[h * BC:(h + 1) * BC, :],
                                     st["gts"][:, h * BC:(h + 1) * BC],
                                     negbwT[:, lt * O:(lt + 1) * O],
                                     start=True, stop=True,
                                     skip_group_check=True)
            if g == 0:
                # corr PSUM -> acc (ScalarE), so the DVE chain reads only one
                # PSUM operand per op
                nc.scalar.copy(acc[:], ptr[:])
            for j in range(4):
                e = 4 * g + j
                pj = pes[j][:, E:E + O] if e == 0 else pes[j][:]
                if e == 0:
                    nc.vector.scalar_tensor_tensor(
                        acc[:], pj, gn[:, 0:1], acc[:],
                        op0=mybir.AluOpType.mult, op1=mybir.AluOpType.add)
                elif e < E - 1:
                    nc.vector.scalar_tensor_tensor(
                        acc[:], pj, gn[:, e:e + 1], acc[:],
                        op0=mybir.AluOpType.mult, op1=mybir.AluOpType.add)
                else:
                    nc.vector.scalar_tensor_tensor(
                        osb[:], pj, gn[:, e:e + 1], acc[:],
                        op0=mybir.AluOpType.mult, op1=mybir.AluOpType.add)
            if g == 1:
                nc.sync.dma_start(out_d[t], osb[:])

        emit_negbw()
        for rep in range(reps):
            for t in range(NT):
                st = tile_state(rep, t, xr=xr0 if (rep == 0 and t == 0) else None)
                ptr = pcor.tile([P, O], F32, tag="pcor", name=f"pc{rep}_{t}")
                for g in range(2):
                    pes = emit_group_mm(rep, t, st, g)
                    emit_group_tail(rep, t, st, g, pes, ptr)

    nc.compile()
    return nc


def _prep_shared(w_gate, expert_w, expert_bias):
    # packed per-chunk weight rows: [gate(8) | expert0(300) | ... | expert7(300)]
    wg_c = w_gate.reshape(KC, 128, E).transpose(1, 0, 2)            # [128,6,8]
    we_c = expert_w.reshape(E, O, KC, 128).transpose(3, 2, 0, 1)    # [128,6,8,300]
    w_host = np.ascontiguousarray(np.concatenate(
        [wg_c, we_c.reshape(128, KC, E * O)], axis=2), dtype=np.float16)
    bias_host = np.ascontiguousarray(
        expert_bias.reshape(E, L, KC, 128).transpose(3, 2, 0, 1),
        dtype=np.float16)
    return w_host, bias_host


def _make_in_maps(inputs):
    x = np.asarray(inputs["x"], dtype=np.float32)
    w_host, bias_host = _prep_shared(
        np.asarray(inputs["w_gate"], dtype=np.float32),
        np.asarray(inputs["expert_w"], dtype=np.float32),
        np.asarray(inputs["expert_bias"], dtype=np.float32))
    in_maps = []
    for c in range(NCORES):
        xc = x[c * BC:(c + 1) * BC]                    # [64, 50, 768]
        xl = xc.transpose(1, 0, 2).reshape(TOK, D)     # l-major tokens
        xt = np.ascontiguousarray(
            xl.reshape(NT, P, KC, 128).transpose(0, 3, 2, 1),
            dtype=np.float16)
        in_maps.append({"xt": xt, "w": w_host, "bias": bias_host})
    return in_maps


def kernel(x, w_gate, expert_w, expert_bias):
    if "nc" not in _CACHE:
        _CACHE["nc"] = _build_nc()
    nc = _CACHE["nc"]

    in_maps = _make_in_maps({"x": x, "w_gate": w_gate, "expert_w": expert_w,
                             "expert_bias": expert_bias})

    res = bass_utils.run_bass_kernel_spmd(nc, in_maps,
                                          core_ids=list(range(NCORES)))

    outs = []
    for c in range(NCORES):
        oc = res.results[c]["out"].reshape(L, BC, O).transpose(1, 0, 2)
        outs.append(oc)
    return np.ascontiguousarray(np.concatenate(outs, axis=0))


if __name__ == "__main__":
    rng = np.random.default_rng(0)
    inputs = {
        "x": rng.standard_normal((B, L, D), dtype=np.float32),
        "w_gate": (rng.standard_normal((D, E)) * 0.02).astype(np.float32),
        "expert_w": (rng.standard_normal((E, O, D)) * 0.02).astype(np.float32),
        "expert_bias": (rng.standard_normal((E, L, D)) * 0.02).astype(np.float32),
    }
    out = kernel(**inputs)
    print("out", out.shape, out.dtype, np.abs(out).mean())



# revision 3
# speedup vs baseline: 1.1754x; 1.1754x over previous
"""Trainium2 Bass kernel for MoEAdaptorLayer (moe_routing).

Reference computation (B=512, L=50, D=768, O=300, E=8):
    gates = softmax(x @ w_gate)                          # [B,L,E]
    xw    = einsum('bli,eoi->bleo', x, expert_w)         # [B,L,E,O]
    bw    = einsum('eli,eoi->leo', expert_bias, expert_w)
    out   = einsum('ble,bleo->blo', gates, xw - bw[None])

Strategy: data-parallel over B across 8 cores (64 batches/core); no
collectives. Tokens are laid out l-major per core (token = l*64 + b), so each
128-token tile covers exactly two l values. Matmul operands are rounded to
fp16 on host (fp32 PSUM accumulation); negbw[e,l,o] = -sum_i bias*W is
precomputed on the host (weight-only preprocessing, 0.2% of total FLOPs) so
the device does no preamble work. Per 128-token tile, emission is expert-major
so each expert's PSUM bank closes as early as possible:
  - e0's matmul packs the gate-logit columns; as soon as its bank closes, ACT
    computes unnormalized exp-gates ghat (+row-sum), DVE the reciprocal rs.
  - PE transposes ghat (into spare columns of the corr PSUM bank) mid-g0; ACT
    casts it to fp16 gts; two K=8 matmuls (one per l-half) compute the
    unnormalized gate-weighted bias correction into the corr bank.
  - the weighted sum over experts is split: ACT does scaled copies
    t_e = ghat_e*P_e for e0-e3 (freeing those banks early) plus
    acci = rs*corr; DVE folds e4-e7 via scalar_tensor_tensor into the t_e,
    merges with a small tree, and the final op applies rs:
    osb = rs*(sum_e ghat_e P_e) + rs*corr.
"""

import sys

sys.path.insert(0, "/opt/trn_rl_repo")

from contextlib import ExitStack

import numpy as np

import concourse.bass as bass  # noqa: F401
import concourse.tile as tile
from concourse import bacc, mybir
from concourse import bass_utils
from concourse.masks import make_identity

B, L, D, O, E = 512, 50, 768, 300, 8
NCORES = 8
BC = B // NCORES          # 64 batches per core
TOK = BC * L              # 3200 tokens per core
P = 128                   # tokens per tile
NT = TOK // P             # 25 tiles per core
KC = D // 128             # 6 contraction chunks
WCOL = E + E * O          # packed w row: [gate(8) | e0(300) | ... | e7(300)]
PTW = 428                 # pcor tile width: corr [0:300] + ghat^T [300:428]

F32 = mybir.dt.float32
FP16 = mybir.dt.float16

_CACHE: dict = {}


def _build_nc():
    nc = bacc.Bacc("TRN2", target_bir_lowering=False, debug=False,
                   num_devices=NCORES)

    xt_d = nc.dram_tensor("xt", [NT, P, KC, 128], FP16, kind="ExternalInput").ap()
    w_d = nc.dram_tensor("w", [128, KC, WCOL], FP16, kind="ExternalInput").ap()
    nbw_d = nc.dram_tensor("nbw", [E, L * O], FP16, kind="ExternalInput").ap()
    out_d = nc.dram_tensor("out", [NT, P, O], F32, kind="ExternalOutput").ap()

    AF = mybir.ActivationFunctionType
    ALU = mybir.AluOpType

    with tile.TileContext(nc) as tc, ExitStack() as ctx:
        const = ctx.enter_context(tc.tile_pool(name="const", bufs=1))
        xpool = ctx.enter_context(tc.tile_pool(name="xpool", bufs=3))
        spool = ctx.enter_context(tc.tile_pool(name="spool", bufs=3))
        tpool = ctx.enter_context(tc.tile_pool(name="tpool", bufs=8))
        apool = ctx.enter_context(tc.tile_pool(name="apool", bufs=14))
        opool = ctx.enter_context(tc.tile_pool(name="opool", bufs=3))
        pexp = ctx.enter_context(tc.tile_pool(name="pexp", bufs=7, space="PSUM"))
        pcor = ctx.enter_context(tc.tile_pool(name="pcor", bufs=1, space="PSUM"))

        # --- constants: params pre-packed on host, fp16 -------------------
        # pre-issue tile-0 x DMA so it overlaps the w DMAs
        xr0 = xpool.tile([P, KC, 128], FP16, tag="xr", name="xr_pre0")
        nc.sync.dma_start(xr0[:], xt_d[0])

        w_sb = []
        for c in range(KC):
            wc = const.tile([128, WCOL], FP16, tag=f"w{c}", name=f"w_sb{c}")
            nc.sync.dma_start(wc[:], w_d[:, c])
            w_sb.append(wc)
        negbw = const.tile([E, L * O], FP16, tag="negbw")
        nc.sync.dma_start(negbw[:], nbw_d[:])

        ident = const.tile([128, 128], F32, tag="ident")
        make_identity(nc, ident[:])

        # --- token tiles --------------------------------------------------
        def emit_tile(t, xr=None):
            if xr is None:
                xr = xpool.tile([P, KC, 128], FP16, tag="xr", name=f"xr{t}")
                nc.sync.dma_start(xr[:], xt_d[t])

            pes = []
            for e in range(E):
                wid = E + O if e == 0 else O
                pes.append(pexp.tile([P, wid], F32, tag="pexp",
                                     name=f"pe{t}_{e}"))
            ptr = pcor.tile([P, PTW], F32, tag="pcor", name=f"pc{t}")

            ghat = spool.tile([P, E], F32, tag="ghat", name=f"ghat{t}")
            gsum = spool.tile([P, 1], F32, tag="gsum", name=f"gsum{t}")
            rs = spool.tile([P, 1], F32, tag="rs", name=f"rs{t}")
            gts = spool.tile([E, P], FP16, tag="gts", name=f"gts{t}")
            acci = spool.tile([P, O], FP16, tag="acci", name=f"acci{t}")
            ts = [tpool.tile([P, O], FP16, tag="tmp", name=f"t{t}_{e}")
                  for e in range(4)]
            a4 = apool.tile([P, O], FP16, tag="stt", name=f"a4_{t}")
            a5 = apool.tile([P, O], FP16, tag="stt", name=f"a5_{t}")
            a6 = apool.tile([P, O], FP16, tag="stt", name=f"a6_{t}")
            a7 = apool.tile([P, O], FP16, tag="stt", name=f"a7_{t}")
            u1 = apool.tile([P, O], FP16, tag="stt", name=f"u1_{t}")
            u2 = apool.tile([P, O], FP16, tag="stt", name=f"u2_{t}")
            v = apool.tile([P, O], FP16, tag="stt", name=f"v_{t}")
            osb = opool.tile([P, O], F32, tag="osb", name=f"osb{t}")

            def mm_expert(e):
                lo = 0 if e == 0 else E + e * O
                wid = pes[e].shape[-1]
                for c in range(KC):
                    nc.tensor.matmul(pes[e][:], xr[:, c, :],
                                     w_sb[c][:, lo:lo + wid],
                                     start=(c == 0), stop=(c == KC - 1))

            # group 0, expert-major; gate path interleaved
            mm_expert(0)
            # unnormalized gates: ghat = exp(logits), gsum = row-sum
            nc.scalar.activation(ghat[:], pes[0][:, 0:E], AF.Exp,
                                 accum_out=gsum[:])
            nc.vector.reciprocal(rs[:], gsum[:])
            mm_expert(1)
            nc.tensor.transpose(ptr[0:E, O:O + 128], ghat[:], ident[:])
            nc.scalar.mul(ts[0][:], pes[0][:, E:E + O], ghat[:, 0:1])
            mm_expert(2)
            nc.scalar.mul(ts[1][:], pes[1][:], ghat[:, 1:2])
            nc.scalar.copy(gts[:], ptr[0:E, O:O + 128])
            mm_expert(3)
            nc.scalar.mul(ts[2][:], pes[2][:], ghat[:, 2:3])
            # unnormalized gate-weighted bias correction, one matmul per
            # l-half:  corr[m,:] = sum_e ghat[m,e] * negbw[l(m),e,:]
            for h in range(2):
                lt = 2 * t + h
                nc.tensor.matmul(ptr[h * BC:(h + 1) * BC, 0:O],
                                 gts[:, h * BC:(h + 1) * BC],
                                 negbw[:, lt * O:(lt + 1) * O],
                                 start=True, stop=True,
                                 skip_group_check=True)
            nc.scalar.mul(ts[3][:], pes[3][:], ghat[:, 3:4])

            # group 1
            mm_expert(4)
            nc.vector.scalar_tensor_tensor(a4[:], pes[4][:], ghat[:, 4:5],
                                           ts[0][:], op0=ALU.mult, op1=ALU.add)
            mm_expert(5)
            nc.vector.scalar_tensor_tensor(a5[:], pes[5][:], ghat[:, 5:6],
                                           ts[1][:], op0=ALU.mult, op1=ALU.add)
            mm_expert(6)
            nc.scalar.mul(acci[:], ptr[:, 0:O], rs[:])
            nc.vector.scalar_tensor_tensor(a6[:], pes[6][:], ghat[:, 6:7],
                                           ts[2][:], op0=ALU.mult, op1=ALU.add)
            mm_expert(7)
            nc.vector.scalar_tensor_tensor(a7[:], pes[7][:], ghat[:, 7:8],
                                           ts[3][:], op0=ALU.mult, op1=ALU.add)
            nc.vector.tensor_add(u1[:], a4[:], a5[:])
            nc.vector.tensor_add(u2[:], a6[:], a7[:])
            nc.vector.tensor_add(v[:], u1[:], u2[:])
            # osb = rs * sum_e ghat_e P_e + rs * corr
            nc.vector.scalar_tensor_tensor(osb[:], v[:], rs[:], acci[:],
                                           op0=ALU.mult, op1=ALU.add)
            nc.sync.dma_start(out_d[t], osb[:])

        for t in range(NT):
            emit_tile(t, xr=xr0 if t == 0 else None)

    nc.compile()
    return nc


def _prep_shared(w_gate, expert_w, expert_bias):
    # packed per-chunk weight rows: [gate(8) | expert0(300) | ... | expert7(300)]
    wg_c = w_gate.reshape(KC, 128, E).transpose(1, 0, 2)            # [128,6,8]
    we_c = expert_w.reshape(E, O, KC, 128).transpose(3, 2, 0, 1)    # [128,6,8,300]
    w_host = np.ascontiguousarray(np.concatenate(
        [wg_c, we_c.reshape(128, KC, E * O)], axis=2), dtype=np.float16)
    # negbw[e, l*O + o] = -sum_i expert_bias[e,l,i] * expert_w[e,o,i]
    nbw = -np.einsum('eli,eoi->elo', expert_bias, expert_w,
                     optimize=True)
    nbw_host = np.ascontiguousarray(
        nbw.reshape(E, L * O), dtype=np.float16)
    return w_host, nbw_host


def _make_in_maps(inputs):
    x = np.asarray(inputs["x"], dtype=np.float32)
    w_host, nbw_host = _prep_shared(
        np.asarray(inputs["w_gate"], dtype=np.float32),
        np.asarray(inputs["expert_w"], dtype=np.float32),
        np.asarray(inputs["expert_bias"], dtype=np.float32))
    in_maps = []
    for c in range(NCORES):
        xc = x[c * BC:(c + 1) * BC]                    # [64, 50, 768]
        xl = xc.transpose(1, 0, 2).reshape(TOK, D)     # l-major tokens
        xt = np.ascontiguousarray(
            xl.reshape(NT, P, KC, 128).transpose(0, 3, 2, 1),
            dtype=np.float16)
        in_maps.append({"xt": xt, "w": w_host, "nbw": nbw_host})
    return in_maps


def kernel(x, w_gate, expert_w, expert_bias):
    if "nc" not in _CACHE:
        _CACHE["nc"] = _build_nc()
    nc = _CACHE["nc"]

    in_maps = _make_in_maps({"x": x, "w_gate": w_gate, "expert_w": expert_w,
                             "expert_bias": expert_bias})

    res = bass_utils.run_bass_kernel_spmd(nc, in_maps,
                                          core_ids=list(range(NCORES)))

    outs = []
    for c in range(NCORES):
        oc = res.results[c]["out"].reshape(L, BC, O).transpose(1, 0, 2)
        outs.append(oc)
    return np.ascontiguousarray(np.concatenate(outs, axis=0))


if __name__ == "__main__":
    rng = np.random.default_rng(0)
    inputs = {
        "x": rng.standard_normal((B, L, D), dtype=np.float32),
        "w_gate": (rng.standard_normal((D, E)) * 0.02).astype(np.float32),
        "expert_w": (rng.standard_normal((E, O, D)) * 0.02).astype(np.float32),
        "expert_bias": (rng.standard_normal((E, L, D)) * 0.02).astype(np.float32),
    }
    out = kernel(**inputs)
    print("out", out.shape, out.dtype, np.abs(out).mean())


# revision 6
# speedup vs baseline: 1.1785x; 1.0027x over previous
"""Trainium2 Bass kernel for MoEAdaptorLayer (moe_routing).

Reference computation (B=512, L=50, D=768, O=300, E=8):
    gates = softmax(x @ w_gate)                          # [B,L,E]
    xw    = einsum('bli,eoi->bleo', x, expert_w)         # [B,L,E,O]
    bw    = einsum('eli,eoi->leo', expert_bias, expert_w)
    out   = einsum('ble,bleo->blo', gates, xw - bw[None])

Strategy: data-parallel over B across 8 cores (64 batches/core); no
collectives. Tokens are laid out l-major per core (token = l*64 + b), so each
128-token tile covers exactly two l values. Matmul operands are rounded to
fp16 on host (fp32 PSUM accumulation); negbw[e,l,o] = -sum_i bias*W is
precomputed on the host (weight-only preprocessing, 0.2% of total FLOPs) so
the device does no preamble work. Per 128-token tile, emission is expert-major
so each expert's PSUM bank closes as early as possible:
  - e0's matmul packs the gate-logit columns; as soon as its bank closes, ACT
    computes unnormalized exp-gates ghat (+row-sum), DVE the reciprocal rs.
  - PE transposes ghat (into spare columns of the corr PSUM bank) mid-g0; ACT
    casts it to fp16 gts; two K=8 matmuls (one per l-half) compute the
    unnormalized gate-weighted bias correction into the corr bank.
  - the weighted sum over experts is split: ACT does scaled copies
    t_e = ghat_e*P_e for e0-e3 (freeing those banks early) plus
    acci = rs*corr; DVE folds e4-e7 via scalar_tensor_tensor into the t_e,
    merges with a small tree, and the final op applies rs:
    osb = rs*(sum_e ghat_e P_e) + rs*corr.
"""

import sys

sys.path.insert(0, "/opt/trn_rl_repo")

from contextlib import ExitStack

import numpy as np

import concourse.bass as bass  # noqa: F401
import concourse.tile as tile
from concourse import bacc, mybir
from concourse import bass_utils
from concourse.masks import make_identity

B, L, D, O, E = 512, 50, 768, 300, 8
NCORES = 8
BC = B // NCORES          # 64 batches per core
TOK = BC * L              # 3200 tokens per core
P = 128                   # tokens per tile
NT = TOK // P             # 25 tiles per core
KC = D // 128             # 6 contraction chunks
WCOL = E + E * O          # packed w row: [gate(8) | e0(300) | ... | e7(300)]
PTW = 428                 # pcor tile width: corr [0:300] + ghat^T [300:428]

F32 = mybir.dt.float32
FP16 = mybir.dt.float16

_CACHE: dict = {}


def _build_nc():
    nc = bacc.Bacc("TRN2", target_bir_lowering=False, debug=False,
                   num_devices=NCORES)

    xt_d = nc.dram_tensor("xt", [NT, P, KC, 128], FP16, kind="ExternalInput").ap()
    w_d = nc.dram_tensor("w", [128, KC, WCOL], FP16, kind="ExternalInput").ap()
    nbw_d = nc.dram_tensor("nbw", [E, L * O], FP16, kind="ExternalInput").ap()
    out_d = nc.dram_tensor("out", [NT, P, O], F32, kind="ExternalOutput").ap()

    AF = mybir.ActivationFunctionType
    ALU = mybir.AluOpType

    with tile.TileContext(nc) as tc, ExitStack() as ctx:
        const = ctx.enter_context(tc.tile_pool(name="const", bufs=1))
        xpool = ctx.enter_context(tc.tile_pool(name="xpool", bufs=3))
        spool = ctx.enter_context(tc.tile_pool(name="spool", bufs=3))
        tpool = ctx.enter_context(tc.tile_pool(name="tpool", bufs=8))
        apool = ctx.enter_context(tc.tile_pool(name="apool", bufs=14))
        opool = ctx.enter_context(tc.tile_pool(name="opool", bufs=3))
        pexp = ctx.enter_context(tc.tile_pool(name="pexp", bufs=7, space="PSUM"))
        pcor = ctx.enter_context(tc.tile_pool(name="pcor", bufs=1, space="PSUM"))

        # --- constants: params pre-packed on host, fp16 -------------------
        # x tiles ride the Sync hardware DGE queue; w + negbw go on the
        # Scalar engine's queue so the two streams transfer concurrently
        # at startup.
        xr0 = xpool.tile([P, KC, 128], FP16, tag="xr", name="xr_pre0")
        nc.sync.dma_start(xr0[:], xt_d[0])

        w_sb = []
        for c in range(KC):
            wc = const.tile([128, WCOL], FP16, tag=f"w{c}", name=f"w_sb{c}")
            nc.scalar.dma_start(wc[:], w_d[:, c])
            w_sb.append(wc)
        negbw = const.tile([E, L * O], FP16, tag="negbw")
        nc.scalar.dma_start(negbw[:], nbw_d[:])

        ident = const.tile([128, 128], F32, tag="ident")
        make_identity(nc, ident[:])

        # --- token tiles --------------------------------------------------
        def emit_tile(t, xr=None):
            if xr is None:
                xr = xpool.tile([P, KC, 128], FP16, tag="xr", name=f"xr{t}")
                nc.sync.dma_start(xr[:], xt_d[t])

            pes = []
            for e in range(E):
                wid = E + O if e == 0 else O
                pes.append(pexp.tile([P, wid], F32, tag="pexp",
                                     name=f"pe{t}_{e}"))
            ptr = pcor.tile([P, PTW], F32, tag="pcor", name=f"pc{t}")

            ghat = spool.tile([P, E], F32, tag="ghat", name=f"ghat{t}")
            gsum = spool.tile([P, 1], F32, tag="gsum", name=f"gsum{t}")
            rs = spool.tile([P, 1], F32, tag="rs", name=f"rs{t}")
            gn = spool.tile([P, E], F32, tag="gn", name=f"gn{t}")
            gts = spool.tile([E, P], FP16, tag="gts", name=f"gts{t}")
            acci = spool.tile([P, O], FP16, tag="acci", name=f"acci{t}")
            ts = [tpool.tile([P, O], FP16, tag="tmp", name=f"t{t}_{e}")
                  for e in range(4)]
            a4 = apool.tile([P, O], FP16, tag="stt", name=f"a4_{t}")
            a5 = apool.tile([P, O], FP16, tag="stt", name=f"a5_{t}")
            a6 = apool.tile([P, O], FP16, tag="stt", name=f"a6_{t}")
            m = apool.tile([P, O], FP16, tag="stt", name=f"m_{t}")
            z1 = apool.tile([P, O], FP16, tag="stt", name=f"z1_{t}")
            z2 = apool.tile([P, O], FP16, tag="stt", name=f"z2_{t}")
            z3 = apool.tile([P, O], FP16, tag="stt", name=f"z3_{t}")
            osb = opool.tile([P, O], F32, tag="osb", name=f"osb{t}")

            def mm(e, c):
                lo = 0 if e == 0 else E + e * O
                nc.tensor.matmul(pes[e][:], xr[:, c, :],
                                 w_sb[c][:, lo:lo + pes[e].shape[-1]],
                                 start=(c == 0), stop=(c == KC - 1))

            def mm_expert(e):
                for c in range(KC):
                    mm(e, c)

            # Gate path + corr, emitted into the PE stream after e1 so the
            # cross-engine hops (ACT exp, ACT cast) never stall the PE.
            def emit_gates():
                # normalized gates: ghat = exp(logits), gn = ghat/sum
                nc.scalar.activation(ghat[:], pes[0][:, 0:E], AF.Exp,
                                     accum_out=gsum[:])
                nc.vector.reciprocal(rs[:], gsum[:])
                nc.vector.tensor_scalar_mul(gn[:], ghat[:], rs[:])

            def emit_corr():
                # gate-weighted bias correction, one matmul per l-half:
                # corr[m,:] = sum_e ghat[m,e] * negbw[l(m),e,:]  (unnorm.)
                for h in range(2):
                    lt = 2 * t + h
                    nc.tensor.matmul(ptr[h * BC:(h + 1) * BC, 0:O],
                                     gts[:, h * BC:(h + 1) * BC],
                                     negbw[:, lt * O:(lt + 1) * O],
                                     start=True, stop=True,
                                     skip_group_check=True)

            def emit_tail():
                # weighted sum: ACT drained e0-e3 into ts[*]; DVE folds
                # e4-e6 and the corr into a merge tree while e5/e6/e7
                # stream, leaving a single STT after the last matmul
                nc.vector.scalar_tensor_tensor(a4[:], pes[4][:], gn[:, 4:5],
                                               ts[0][:], op0=ALU.mult,
                                               op1=ALU.add)
                nc.scalar.mul(acci[:], ptr[:, 0:O], rs[:])
                if t != 0:
                    mm_expert(5)
                nc.vector.scalar_tensor_tensor(a5[:], pes[5][:], gn[:, 5:6],
                                               ts[1][:], op0=ALU.mult,
                                               op1=ALU.add)
                nc.vector.tensor_add(m[:], ts[3][:], acci[:])
                nc.vector.tensor_add(z1[:], a4[:], a5[:])
                if t != 0:
                    mm_expert(6)
                nc.vector.scalar_tensor_tensor(a6[:], pes[6][:], gn[:, 6:7],
                                               ts[2][:], op0=ALU.mult,
                                               op1=ALU.add)
                nc.vector.tensor_add(z2[:], a6[:], m[:])
                nc.vector.tensor_add(z3[:], z1[:], z2[:])
                if t != 0:
                    mm_expert(7)
                nc.vector.scalar_tensor_tensor(osb[:], pes[7][:], gn[:, 7:8],
                                               z3[:], op0=ALU.mult,
                                               op1=ALU.add)
                nc.sync.dma_start(out_d[t], osb[:])

            if t == 0:
                # startup tile: chunk-major over e0..e6 so the PE streams
                # each w chunk as its DMA lands (e7 waits for a free bank)
                for c in range(KC):
                    for e in range(7):
                        mm(e, c)
                emit_gates()
                nc.tensor.transpose(ptr[0:E, O:O + 128], ghat[:], ident[:])
                nc.scalar.mul(ts[0][:], pes[0][:, E:E + O], gn[:, 0:1])
                nc.scalar.copy(gts[:], ptr[0:E, O:O + 128])
                nc.scalar.mul(ts[1][:], pes[1][:], gn[:, 1:2])
                emit_corr()
                nc.scalar.mul(ts[2][:], pes[2][:], gn[:, 2:3])
                nc.scalar.mul(ts[3][:], pes[3][:], gn[:, 3:4])
                mm_expert(7)
                emit_tail()
            else:
                mm_expert(0)
                emit_gates()
                mm_expert(1)
                nc.tensor.transpose(ptr[0:E, O:O + 128], ghat[:], ident[:])
                nc.scalar.mul(ts[0][:], pes[0][:, E:E + O], gn[:, 0:1])
                nc.scalar.copy(gts[:], ptr[0:E, O:O + 128])
                mm_expert(2)
                nc.scalar.mul(ts[1][:], pes[1][:], gn[:, 1:2])
                mm_expert(3)
                nc.scalar.mul(ts[2][:], pes[2][:], gn[:, 2:3])
                emit_corr()
                nc.scalar.mul(ts[3][:], pes[3][:], gn[:, 3:4])
                mm_expert(4)
                emit_tail()

        for t in range(NT):
            emit_tile(t, xr=xr0 if t == 0 else None)

    nc.compile()
    return nc


def _prep_shared(w_gate, expert_w, expert_bias):
    # packed per-chunk weight rows: [gate(8) | expert0(300) | ... | expert7(300)]
    wg_c = w_gate.reshape(KC, 128, E).transpose(1, 0, 2)            # [128,6,8]
    we_c = expert_w.reshape(E, O, KC, 128).transpose(3, 2, 0, 1)    # [128,6,8,300]
    w_host = np.ascontiguousarray(np.concatenate(
        [wg_c, we_c.reshape(128, KC, E * O)], axis=2), dtype=np.float16)
    # negbw[e, l*O + o] = -sum_i expert_bias[e,l,i] * expert_w[e,o,i]
    nbw = -np.einsum('eli,eoi->elo', expert_bias, expert_w,
                     optimize=True)
    nbw_host = np.ascontiguousarray(
        nbw.reshape(E, L * O), dtype=np.float16)
    return w_host, nbw_host


def _make_in_maps(inputs):
    x = np.asarray(inputs["x"], dtype=np.float32)
    w_host, nbw_host = _prep_shared(
        np.asarray(inputs["w_gate"], dtype=np.float32),
        np.asarray(inputs["expert_w"], dtype=np.float32),
        np.asarray(inputs["expert_bias"], dtype=np.float32))
    in_maps = []
    for c in range(NCORES):
        xc = x[c * BC:(c + 1) * BC]                    # [64, 50, 768]
        xl = xc.transpose(1, 0, 2).reshape(TOK, D)     # l-major tokens
        xt = np.ascontiguousarray(
            xl.reshape(NT, P, KC, 128).transpose(0, 3, 2, 1),
            dtype=np.float16)
        in_maps.append({"xt": xt, "w": w_host, "nbw": nbw_host})
    return in_maps


def kernel(x, w_gate, expert_w, expert_bias):
    if "nc" not in _CACHE:
        _CACHE["nc"] = _build_nc()
    nc = _CACHE["nc"]

    in_maps = _make_in_maps({"x": x, "w_gate": w_gate, "expert_w": expert_w,
                             "expert_bias": expert_bias})

    res = bass_utils.run_bass_kernel_spmd(nc, in_maps,
                                          core_ids=list(range(NCORES)))

    outs = []
    for c in range(NCORES):
        oc = res.results[c]["out"].reshape(L, BC, O).transpose(1, 0, 2)
        outs.append(oc)
    return np.ascontiguousarray(np.concatenate(outs, axis=0))


if __name__ == "__main__":
    rng = np.random.default_rng(0)
    inputs = {
        "x": rng.standard_normal((B, L, D), dtype=np.float32),
        "w_gate": (rng.standard_normal((D, E)) * 0.02).astype(np.float32),
        "expert_w": (rng.standard_normal((E, O, D)) * 0.02).astype(np.float32),
        "expert_bias": (rng.standard_normal((E, L, D)) * 0.02).astype(np.float32),
    }
    out = kernel(**inputs)
    print("out", out.shape, out.dtype, np.abs(out).mean())
